# revision 26
# baseline (speedup 1.0000x reference)
"""Trainium2 Bass kernel for nn_DRAM_MAC_temporal_encoding (polynomial attention).

Math (QK_mul=1):
    out = sum_i coef_i * (x @ (y-OFF)^i) * decay
        = (x * decay) @ P(y-OFF)            # P = Horner cubic, elementwise
so the whole problem is ONE [S,64]@[64,S] matmul per (b,h) head plus the
output write -> memory-bound. The tiny elementwise prep (poly on y,
row-scaling x, transposes, fp16 casts) runs on host; the device does
matmuls + store.

Precision: tolerance is rel_err < 2e-2. fp16 inputs + single fp16 matmul
(fp32 PSUM accumulate) + fp16 output measures 2.5e-4 on the numpy model —
so no hi/lo split and, crucially, the 50 MiB/core fp32 output write
becomes 25 MiB fp16 (host upcasts back to fp32). PSUM->SBUF fp32->fp16
drains rotate across Vector/Scalar/Pool so no single engine bottlenecks.

QK_mul=0: out = sum_i coef_i * ((x*d^i) @ (y-OFF)^i) -> two K=128 chunks
(4 stacked K=64 terms), same kernel with n_chunks=2.

Sharding: 24 (b,h) heads -> 3 per core across 8 cores.
"""

import ml_dtypes
import numpy as np

import concourse.mybir as mybir
import concourse.tile as tile
from concourse import bacc
from concourse.bass_utils import run_bass_kernel_spmd

C = [0.17393044, 0.15653739, 0.14088365, 0.12679529, 5.51975209,
     4.96777688, 4.4709992, -1.44776001, -1.30298401, 46.05483778]
MAX_ORDER = 3
X_MAX = 0.9
OFFSET = 0.45

B, H, S, D = 2, 12, 2048, 64
BH = B * H
N_CORES = 8
BLK = BH // N_CORES  # heads per core

M_TILE = 128   # output rows per matmul (PSUM partitions)
N_TILE = 512   # output cols per matmul (one fp32 PSUM bank)

_NC_CACHE = {}
_last_nc = None
_last_in_maps = None


def _coefs():
    cs = []
    idx = 0
    for i in range(MAX_ORDER + 1):
        n_j = MAX_ORDER - i + 1
        cs.append(sum(C[idx + j] * X_MAX ** j for j in range(n_j)))
        idx += n_j
    return cs  # [c0, c1, c2, c3]


def _build_nc(n_chunks, wk):
    """Device kernel: per core, BLK independent [S,S] fp16 output blocks,
    each output tile = sum over n_chunks K=128 bf16 matmuls.

    K=64 matmuls stream at ~1/3 the K=128 rate on TRN2 HW (630ns vs 233ns
    per [128,512]), so the contraction is always presented as K=128. Head 0
    uploads full pre-padded 128-row operands ([a_hi; a_lo] and [w; w]) so
    its first matmul isn't gated by the ~1.8us-per-tile Pool memsets; heads
    1+ upload only 64 real rows and zero rows 64:128 on the (otherwise
    idle) Pool engine well before they're needed."""
    nc = bacc.Bacc(None, target_bir_lowering=False)
    a0_d = nc.dram_tensor("a0", [n_chunks, 128, S], mybir.dt.bfloat16,
                          kind="ExternalInput")
    w0_d = nc.dram_tensor("w0", [n_chunks, 128, S], mybir.dt.bfloat16,
                          kind="ExternalInput")
    a_d = nc.dram_tensor("a", [BLK - 1, n_chunks, wk, S], mybir.dt.bfloat16,
                         kind="ExternalInput")
    w_d = nc.dram_tensor("w", [BLK - 1, n_chunks, wk, S], mybir.dt.bfloat16,
                         kind="ExternalInput")
    out_d = nc.dram_tensor("out", [BLK, S, S], mybir.dt.float16,
                           kind="ExternalOutput")

    with tile.TileContext(nc) as tc:
        with (
            tc.tile_pool(name="inp", bufs=1) as inp,
            tc.tile_pool(name="ps", bufs=4, space="PSUM") as psp,
            tc.tile_pool(name="outp", bufs=10) as outp,
        ):
            # Input tiles: zero rows wk:128 are memset up front (blk0's w on
            # DVE, which is idle until drains start; the rest on Pool), and
            # each head's loads are emitted just before its row-tiles so the
            # single DMA FIFO starts storing after only one head's loads.
            a_ts, w_ts = {}, {}
            for blk in range(BLK):
                for c in range(n_chunks):
                    ta = inp.tile([128, S], mybir.dt.bfloat16,
                                  name=f"a{blk}_{c}", tag=f"a{blk}_{c}")
                    a_ts[(blk, c)] = ta
                    tw = inp.tile([128, S], mybir.dt.bfloat16,
                                  name=f"w{blk}_{c}", tag=f"w{blk}_{c}")
                    w_ts[(blk, c)] = tw
            if wk < 128:
                for blk in range(1, BLK):
                    for c in range(n_chunks):
                        nc.gpsimd.memset(a_ts[(blk, c)][wk:], 0.0)
                        nc.gpsimd.memset(w_ts[(blk, c)][wk:], 0.0)

            # Pool/GpSimd can't read PSUM on TRN2, so drains go to DVE and
            # Act. A 2-deep ring of [128,2048] PSUM tiles serializes on the
            # ~2.2us whole-tile drain (measured ~2.0us/row-tile cadence);
            # instead use a 4-deep ring of [128,1024] half-tiles whose
            # ~1.1us drains alternate engines.
            HALF = S // 2
            di = 0
            with nc.allow_low_precision(reason="fp16 out within 2e-2 tol"):
                for blk in range(BLK):
                    for c in range(n_chunks):
                        if blk == 0:
                            nc.sync.dma_start(a_ts[(blk, c)][:], a0_d[c])
                            nc.sync.dma_start(w_ts[(blk, c)][:], w0_d[c])
                        else:
                            nc.sync.dma_start(a_ts[(blk, c)][:wk],
                                              a_d[blk - 1, c])
                            nc.sync.dma_start(w_ts[(blk, c)][:wk],
                                              w_d[blk - 1, c])
                    for st in range(S // M_TILE):
                        ot = outp.tile([M_TILE, S], mybir.dt.float16,
                                       tag="ot")
                        for h in range(2):
                            ps = psp.tile([M_TILE, HALF], mybir.dt.float32,
                                          tag="ps")
                            for ntl in range(HALF // N_TILE):
                                nt = h * (HALF // N_TILE) + ntl
                                for c in range(n_chunks):
                                    nc.tensor.matmul(
                                        ps[:, ntl * N_TILE:
                                           (ntl + 1) * N_TILE],
                                        a_ts[(blk, c)][
                                            :, st * M_TILE:(st + 1) * M_TILE],
                                        w_ts[(blk, c)][
                                            :, nt * N_TILE:(nt + 1) * N_TILE],
                                        start=(c == 0),
                                        stop=(c == n_chunks - 1),
                                    )
                            dst = ot[:, h * HALF:(h + 1) * HALF]
                            if di % 2 == 0:
                                nc.vector.tensor_copy(dst, ps[:])
                            else:
                                nc.scalar.copy(dst, ps[:])
                            di += 1
                        nc.sync.dma_start(
                            out_d[blk, st * M_TILE:(st + 1) * M_TILE, :],
                            ot[:])
    nc.compile()
    return nc


def _get_nc(n_chunks, wk):
    key = (n_chunks, wk)
    if key not in _NC_CACHE:
        _NC_CACHE[key] = _build_nc(n_chunks, wk)
    return _NC_CACHE[key]


def _prepare(x, y, dm, qk):
    """Host prep -> bf16 arrays: a/w [BH, n_chunks, wk, S] (heads 1+ of
    each core) and a0p/w0p [N_CORES, n_chunks, 128, S] (pre-padded head 0
    of each core: [a_hi; a_lo] against [w; w])."""
    c0, c1, c2, c3 = _coefs()
    yo = (y - OFFSET).astype(np.float32)  # [B,H,D,S]
    if qk:
        n_chunks, wk = 1, D
        af = np.ascontiguousarray(
            (x * dm[None, None, :, :]).transpose(0, 1, 3, 2)
        ).reshape(BH, 1, D, S).astype(np.float32)
        a = af.astype(ml_dtypes.bfloat16)
        w = (((c3 * yo + c2) * yo + c1) * yo + c0) \
            .astype(ml_dtypes.bfloat16).reshape(BH, 1, D, S)
        h0 = [c * BLK for c in range(N_CORES)]
        a_hi = a[h0]
        a_lo = (af[h0] - a_hi.astype(np.float32)).astype(ml_dtypes.bfloat16)
        a0p = np.concatenate([a_hi, a_lo], axis=2)
        w0p = np.concatenate([w[h0], w[h0]], axis=2)
    else:
        n_chunks, wk = 2, 2 * D
        d = dm[:, 0]
        a = np.empty((BH, 2, 2 * D, S), dtype=ml_dtypes.bfloat16)
        w = np.empty((BH, 2, 2 * D, S), dtype=ml_dtypes.bfloat16)
        xt = x.transpose(0, 1, 3, 2).reshape(BH, D, S)
        di = np.ones_like(d)
        yi = np.ones_like(yo).reshape(BH, D, S)
        yo_r = yo.reshape(BH, D, S)
        for i, ci in enumerate((c0, c1, c2, c3)):
            c, half = divmod(i, 2)
            a[:, c, half * D:(half + 1) * D] = xt * di[None, None, :]
            w[:, c, half * D:(half + 1) * D] = ci * yi
            di = di * d
            yi = yi * yo_r
        h0 = [c * BLK for c in range(N_CORES)]
        a0p, w0p = a[h0], w[h0]
    return a, w, a0p, w0p, n_chunks, wk


def kernel(**inputs):
    x = np.asarray(inputs["x"], dtype=np.float32)
    y = np.asarray(inputs["y"], dtype=np.float32)
    dm = np.asarray(inputs["decay_mask"], dtype=np.float32)
    qk = int(np.asarray(inputs["QK_mul"]))

    a, w, a0p, w0p, n_chunks, wk = _prepare(x, y, dm, qk)
    nc = _get_nc(n_chunks, wk)

    in_maps = [
        {"a0": a0p[c], "w0": w0p[c],
         "a": a[c * BLK + 1:(c + 1) * BLK],
         "w": w[c * BLK + 1:(c + 1) * BLK]}
        for c in range(N_CORES)
    ]
    global _last_nc, _last_in_maps
    _last_nc, _last_in_maps = nc, in_maps

    res = None
    for attempt in range(3):
        try:
            res = run_bass_kernel_spmd(nc, in_maps,
                                       core_ids=list(range(N_CORES)))
            break
        except Exception:
            # transient NRT_EXEC_UNIT_UNRECOVERABLE wedges occur on busy axon
            # terminals; they clear after a pause
            if attempt == 2:
                raise
            import time
            time.sleep(45)

    out = np.empty((BH, S, S), dtype=np.float32)
    for c in range(N_CORES):
        out[c * BLK:(c + 1) * BLK] = res.results[c]["out"]
    return out.reshape(B, H, S, S)


# revision 29
# speedup vs baseline: 1.0585x; 1.0585x over previous
"""Trainium2 Bass kernel for nn_DRAM_MAC_temporal_encoding (polynomial attention).

Math (QK_mul=1):
    out = sum_i coef_i * (x @ (y-OFF)^i) * decay
        = (x * decay) @ P(y-OFF)            # P = Horner cubic, elementwise
so the whole problem is ONE [S,64]@[64,S] matmul per (b,h) head plus the
output write -> memory-bound. The tiny elementwise prep (poly on y,
row-scaling x, transposes, fp16 casts) runs on host; the device does
matmuls + store.

Precision: tolerance is rel_err < 2e-2. fp16 inputs + single fp16 matmul
(fp32 PSUM accumulate) + fp16 output measures 2.5e-4 on the numpy model —
so no hi/lo split and, crucially, the 50 MiB/core fp32 output write
becomes 25 MiB fp16 (host upcasts back to fp32). PSUM->SBUF fp32->fp16
drains rotate across Vector/Scalar/Pool so no single engine bottlenecks.

QK_mul=0: out = sum_i coef_i * ((x*d^i) @ (y-OFF)^i) -> two K=128 chunks
(4 stacked K=64 terms), same kernel with n_chunks=2.

Sharding: 24 (b,h) heads -> 3 per core across 8 cores.
"""

import ml_dtypes
import numpy as np

import concourse.mybir as mybir
import concourse.tile as tile
from concourse import bacc
from concourse.bass_utils import run_bass_kernel_spmd

C = [0.17393044, 0.15653739, 0.14088365, 0.12679529, 5.51975209,
     4.96777688, 4.4709992, -1.44776001, -1.30298401, 46.05483778]
MAX_ORDER = 3
X_MAX = 0.9
OFFSET = 0.45

B, H, S, D = 2, 12, 2048, 64
BH = B * H
N_CORES = 8
BLK = BH // N_CORES  # heads per core

M_TILE = 128   # output rows per matmul (PSUM partitions)
N_TILE = 512   # output cols per matmul (one fp32 PSUM bank)

_NC_CACHE = {}
_last_nc = None
_last_in_maps = None


def _coefs():
    cs = []
    idx = 0
    for i in range(MAX_ORDER + 1):
        n_j = MAX_ORDER - i + 1
        cs.append(sum(C[idx + j] * X_MAX ** j for j in range(n_j)))
        idx += n_j
    return cs  # [c0, c1, c2, c3]


def _build_nc(n_chunks, wk):
    """Device kernel: per core, BLK independent [S,S] fp16 output blocks,
    each output tile = sum over n_chunks K=128 bf16 matmuls.

    K=64 matmuls stream at ~1/3 the K=128 rate on TRN2 HW (630ns vs 233ns
    per [128,512]), so the contraction is always presented as K=128. Head 0
    uploads full pre-padded 128-row operands ([a_hi; a_lo] and [w; w]) so
    its first matmul isn't gated by the ~1.8us-per-tile Pool memsets; heads
    1+ upload only 64 real rows and zero rows 64:128 on the (otherwise
    idle) Pool engine well before they're needed."""
    nc = bacc.Bacc(None, target_bir_lowering=False)
    aw0_d = nc.dram_tensor("aw0", [n_chunks, 128, 2 * S], mybir.dt.bfloat16,
                           kind="ExternalInput")
    aw_d = nc.dram_tensor("aw", [BLK - 1, n_chunks, wk, 2 * S],
                          mybir.dt.bfloat16, kind="ExternalInput")
    out_d = nc.dram_tensor("out", [BLK, S, S], mybir.dt.float16,
                           kind="ExternalOutput")

    with tile.TileContext(nc) as tc:
        with (
            tc.tile_pool(name="inp", bufs=1) as inp,
            tc.tile_pool(name="ps", bufs=4, space="PSUM") as psp,
            tc.tile_pool(name="outp", bufs=10) as outp,
        ):
            # Input tiles: zero rows wk:128 are memset up front (blk0's w on
            # DVE, which is idle until drains start; the rest on Pool), and
            # each head's loads are emitted just before its row-tiles so the
            # single DMA FIFO starts storing after only one head's loads.
            aw_ts = {}
            for blk in range(BLK):
                for c in range(n_chunks):
                    t = inp.tile([128, 2 * S], mybir.dt.bfloat16,
                                 name=f"aw{blk}_{c}", tag=f"aw{blk}_{c}")
                    aw_ts[(blk, c)] = t
            if wk < 128:
                for blk in range(1, BLK):
                    for c in range(n_chunks):
                        nc.gpsimd.memset(aw_ts[(blk, c)][wk:], 0.0)

            # Pool/GpSimd can't read PSUM on TRN2, so drains go to DVE and
            # Act. A 2-deep ring of [128,2048] PSUM tiles serializes on the
            # ~2.2us whole-tile drain (measured ~2.0us/row-tile cadence);
            # instead use a 4-deep ring of [128,1024] half-tiles whose
            # ~1.1us drains alternate engines.
            HALF = S // 2
            di = 0
            with nc.allow_low_precision(reason="fp16 out within 2e-2 tol"):
                for blk in range(BLK):
                    for c in range(n_chunks):
                        if blk == 0:
                            nc.sync.dma_start(aw_ts[(blk, c)][:], aw0_d[c])
                        else:
                            nc.sync.dma_start(aw_ts[(blk, c)][:wk],
                                              aw_d[blk - 1, c])
                    for st in range(S // M_TILE):
                        ot = outp.tile([M_TILE, S], mybir.dt.float16,
                                       tag="ot")
                        for h in range(2):
                            ps = psp.tile([M_TILE, HALF], mybir.dt.float32,
                                          tag="ps")
                            for ntl in range(HALF // N_TILE):
                                nt = h * (HALF // N_TILE) + ntl
                                for c in range(n_chunks):
                                    nc.tensor.matmul(
                                        ps[:, ntl * N_TILE:
                                           (ntl + 1) * N_TILE],
                                        aw_ts[(blk, c)][
                                            :, st * M_TILE:(st + 1) * M_TILE],
                                        aw_ts[(blk, c)][
                                            :, S + nt * N_TILE:
                                            S + (nt + 1) * N_TILE],
                                        start=(c == 0),
                                        stop=(c == n_chunks - 1),
                                    )
                            dst = ot[:, h * HALF:(h + 1) * HALF]
                            if di % 2 == 0:
                                nc.vector.tensor_copy(dst, ps[:])
                            else:
                                nc.scalar.copy(dst, ps[:])
                            di += 1
                        nc.sync.dma_start(
                            out_d[blk, st * M_TILE:(st + 1) * M_TILE, :],
                            ot[:])
    nc.compile()
    return nc


def _get_nc(n_chunks, wk):
    key = (n_chunks, wk)
    if key not in _NC_CACHE:
        _NC_CACHE[key] = _build_nc(n_chunks, wk)
    return _NC_CACHE[key]


def _prepare(x, y, dm, qk):
    """Host prep -> bf16 arrays: aw [BH, n_chunks, wk, 2S] (a|w column
    blocks, heads 1+ of each core) and aw0p [N_CORES, n_chunks, 128, 2S]
    (pre-padded head 0 of each core: [a_hi; a_lo] | [w; w])."""
    c0, c1, c2, c3 = _coefs()
    yo = (y - OFFSET).astype(np.float32)  # [B,H,D,S]
    if qk:
        n_chunks, wk = 1, D
        af = np.ascontiguousarray(
            (x * dm[None, None, :, :]).transpose(0, 1, 3, 2)
        ).reshape(BH, 1, D, S).astype(np.float32)
        aw = np.empty((BH, 1, D, 2 * S), dtype=ml_dtypes.bfloat16)
        aw[..., :S] = af
        aw[..., S:] = (((c3 * yo + c2) * yo + c1) * yo + c0) \
            .astype(ml_dtypes.bfloat16).reshape(BH, 1, D, S)
        h0 = [c * BLK for c in range(N_CORES)]
        aw0p = np.empty((N_CORES, 1, 128, 2 * S), dtype=ml_dtypes.bfloat16)
        aw0p[:, :, :D] = aw[h0]
        aw0p[:, :, D:, :S] = (af[h0] - aw[h0][..., :S].astype(np.float32)) \
            .astype(ml_dtypes.bfloat16)
        aw0p[:, :, D:, S:] = aw[h0][..., S:]
    else:
        n_chunks, wk = 2, 2 * D
        d = dm[:, 0]
        aw = np.empty((BH, 2, 2 * D, 2 * S), dtype=ml_dtypes.bfloat16)
        xt = x.transpose(0, 1, 3, 2).reshape(BH, D, S)
        di = np.ones_like(d)
        yi = np.ones_like(yo).reshape(BH, D, S)
        yo_r = yo.reshape(BH, D, S)
        for i, ci in enumerate((c0, c1, c2, c3)):
            c, half = divmod(i, 2)
            aw[:, c, half * D:(half + 1) * D, :S] = xt * di[None, None, :]
            aw[:, c, half * D:(half + 1) * D, S:] = ci * yi
            di = di * d
            yi = yi * yo_r
        h0 = [c * BLK for c in range(N_CORES)]
        aw0p = np.ascontiguousarray(aw[h0])
    return aw, aw0p, n_chunks, wk


def kernel(**inputs):
    x = np.asarray(inputs["x"], dtype=np.float32)
    y = np.asarray(inputs["y"], dtype=np.float32)
    dm = np.asarray(inputs["decay_mask"], dtype=np.float32)
    qk = int(np.asarray(inputs["QK_mul"]))

    aw, aw0p, n_chunks, wk = _prepare(x, y, dm, qk)
    nc = _get_nc(n_chunks, wk)

    in_maps = [
        {"aw0": aw0p[c], "aw": aw[c * BLK + 1:(c + 1) * BLK]}
        for c in range(N_CORES)
    ]
    global _last_nc, _last_in_maps
    _last_nc, _last_in_maps = nc, in_maps

    res = None
    for attempt in range(3):
        try:
            res = run_bass_kernel_spmd(nc, in_maps,
                                       core_ids=list(range(N_CORES)))
            break
        except Exception:
            # transient NRT_EXEC_UNIT_UNRECOVERABLE wedges occur on busy axon
            # terminals; they clear after a pause
            if attempt == 2:
                raise
            import time
            time.sleep(45)

    out = np.empty((BH, S, S), dtype=np.float32)
    for c in range(N_CORES):
        out[c * BLK:(c + 1) * BLK] = res.results[c]["out"]
    return out.reshape(B, H, S, S)


# revision 30
# speedup vs baseline: 1.0734x; 1.0141x over previous
"""Trainium2 Bass kernel for nn_DRAM_MAC_temporal_encoding (polynomial attention).

Math (QK_mul=1):
    out = sum_i coef_i * (x @ (y-OFF)^i) * decay
        = (x * decay) @ P(y-OFF)            # P = Horner cubic, elementwise
so the whole problem is ONE [S,64]@[64,S] matmul per (b,h) head plus the
output write -> memory-bound. The tiny elementwise prep (poly on y,
row-scaling x, transposes, fp16 casts) runs on host; the device does
matmuls + store.

Precision: tolerance is rel_err < 2e-2. fp16 inputs + single fp16 matmul
(fp32 PSUM accumulate) + fp16 output measures 2.5e-4 on the numpy model —
so no hi/lo split and, crucially, the 50 MiB/core fp32 output write
becomes 25 MiB fp16 (host upcasts back to fp32). PSUM->SBUF fp32->fp16
drains rotate across Vector/Scalar/Pool so no single engine bottlenecks.

QK_mul=0: out = sum_i coef_i * ((x*d^i) @ (y-OFF)^i) -> two K=128 chunks
(4 stacked K=64 terms), same kernel with n_chunks=2.

Sharding: 24 (b,h) heads -> 3 per core across 8 cores.
"""

import ml_dtypes
import numpy as np

import concourse.mybir as mybir
import concourse.tile as tile
from concourse import bacc
from concourse.bass_utils import run_bass_kernel_spmd

C = [0.17393044, 0.15653739, 0.14088365, 0.12679529, 5.51975209,
     4.96777688, 4.4709992, -1.44776001, -1.30298401, 46.05483778]
MAX_ORDER = 3
X_MAX = 0.9
OFFSET = 0.45

B, H, S, D = 2, 12, 2048, 64
BH = B * H
N_CORES = 8
BLK = BH // N_CORES  # heads per core

M_TILE = 128   # output rows per matmul (PSUM partitions)
N_TILE = 512   # output cols per matmul (one fp32 PSUM bank)

_NC_CACHE = {}
_last_nc = None
_last_in_maps = None


def _coefs():
    cs = []
    idx = 0
    for i in range(MAX_ORDER + 1):
        n_j = MAX_ORDER - i + 1
        cs.append(sum(C[idx + j] * X_MAX ** j for j in range(n_j)))
        idx += n_j
    return cs  # [c0, c1, c2, c3]


def _build_nc(n_chunks, wk):
    """Device kernel: per core, BLK independent [S,S] fp16 output blocks,
    each output tile = sum over n_chunks K=128 bf16 matmuls.

    K=64 matmuls stream at ~1/3 the K=128 rate on TRN2 HW (630ns vs 233ns
    per [128,512]), so the contraction is always presented as K=128. Head 0
    uploads full pre-padded 128-row operands ([a_hi; a_lo] and [w; w]) so
    its first matmul isn't gated by the ~1.8us-per-tile Pool memsets; heads
    1+ upload only 64 real rows and zero rows 64:128 on the (otherwise
    idle) Pool engine well before they're needed."""
    nc = bacc.Bacc(None, target_bir_lowering=False)
    aw_d = nc.dram_tensor("aw", [BLK, n_chunks, wk, 2 * S],
                          mybir.dt.bfloat16, kind="ExternalInput")
    out_d = nc.dram_tensor("out", [BLK, S, S], mybir.dt.float16,
                           kind="ExternalOutput")

    with tile.TileContext(nc) as tc:
        with (
            tc.tile_pool(name="inp", bufs=1) as inp,
            tc.tile_pool(name="ps", bufs=4, space="PSUM") as psp,
            tc.tile_pool(name="outp", bufs=10) as outp,
        ):
            # Input tiles: zero rows wk:128 are memset up front (blk0's w on
            # DVE, which is idle until drains start; the rest on Pool), and
            # each head's loads are emitted just before its row-tiles so the
            # single DMA FIFO starts storing after only one head's loads.
            aw_ts = {}
            for blk in range(BLK):
                for c in range(n_chunks):
                    t = inp.tile([128, 2 * S], mybir.dt.bfloat16,
                                 name=f"aw{blk}_{c}", tag=f"aw{blk}_{c}")
                    aw_ts[(blk, c)] = t
            if wk < 128:
                for blk in range(BLK):
                    for c in range(n_chunks):
                        if blk == 0:
                            # head 0 gates the pipeline start: its zero rows
                            # go to DVE/Act (idle until the first drains,
                            # and 2x-mode-fast on bf16) instead of Pool
                            nc.vector.memset(aw_ts[(blk, c)][wk:, :S], 0.0)
                            nc.scalar.memzero(aw_ts[(blk, c)][wk:, S:])
                        else:
                            nc.gpsimd.memset(aw_ts[(blk, c)][wk:], 0.0)

            # Pool/GpSimd can't read PSUM on TRN2, so drains go to DVE and
            # Act. A 2-deep ring of [128,2048] PSUM tiles serializes on the
            # ~2.2us whole-tile drain (measured ~2.0us/row-tile cadence);
            # instead use a 4-deep ring of [128,1024] half-tiles whose
            # ~1.1us drains alternate engines.
            HALF = S // 2
            di = 0
            with nc.allow_low_precision(reason="fp16 out within 2e-2 tol"):
                for blk in range(BLK):
                    for c in range(n_chunks):
                        nc.sync.dma_start(aw_ts[(blk, c)][:wk],
                                          aw_d[blk, c])
                    for st in range(S // M_TILE):
                        ot = outp.tile([M_TILE, S], mybir.dt.float16,
                                       tag="ot")
                        for h in range(2):
                            ps = psp.tile([M_TILE, HALF], mybir.dt.float32,
                                          tag="ps")
                            for ntl in range(HALF // N_TILE):
                                nt = h * (HALF // N_TILE) + ntl
                                for c in range(n_chunks):
                                    nc.tensor.matmul(
                                        ps[:, ntl * N_TILE:
                                           (ntl + 1) * N_TILE],
                                        aw_ts[(blk, c)][
                                            :, st * M_TILE:(st + 1) * M_TILE],
                                        aw_ts[(blk, c)][
                                            :, S + nt * N_TILE:
                                            S + (nt + 1) * N_TILE],
                                        start=(c == 0),
                                        stop=(c == n_chunks - 1),
                                    )
                            dst = ot[:, h * HALF:(h + 1) * HALF]
                            if di % 2 == 0:
                                nc.vector.tensor_copy(dst, ps[:])
                            else:
                                nc.scalar.copy(dst, ps[:])
                            di += 1
                        nc.sync.dma_start(
                            out_d[blk, st * M_TILE:(st + 1) * M_TILE, :],
                            ot[:])
    nc.compile()
    return nc


def _get_nc(n_chunks, wk):
    key = (n_chunks, wk)
    if key not in _NC_CACHE:
        _NC_CACHE[key] = _build_nc(n_chunks, wk)
    return _NC_CACHE[key]


def _prepare(x, y, dm, qk):
    """Host prep -> bf16 arrays: aw [BH, n_chunks, wk, 2S] (a|w column
    blocks, heads 1+ of each core) and aw0p [N_CORES, n_chunks, 128, 2S]
    (pre-padded head 0 of each core: [a_hi; a_lo] | [w; w])."""
    c0, c1, c2, c3 = _coefs()
    yo = (y - OFFSET).astype(np.float32)  # [B,H,D,S]
    if qk:
        n_chunks, wk = 1, D
        af = np.ascontiguousarray(
            (x * dm[None, None, :, :]).transpose(0, 1, 3, 2)
        ).reshape(BH, 1, D, S).astype(np.float32)
        aw = np.empty((BH, 1, D, 2 * S), dtype=ml_dtypes.bfloat16)
        aw[..., :S] = af
        aw[..., S:] = (((c3 * yo + c2) * yo + c1) * yo + c0) \
            .astype(ml_dtypes.bfloat16).reshape(BH, 1, D, S)
    else:
        n_chunks, wk = 2, 2 * D
        d = dm[:, 0]
        aw = np.empty((BH, 2, 2 * D, 2 * S), dtype=ml_dtypes.bfloat16)
        xt = x.transpose(0, 1, 3, 2).reshape(BH, D, S)
        di = np.ones_like(d)
        yi = np.ones_like(yo).reshape(BH, D, S)
        yo_r = yo.reshape(BH, D, S)
        for i, ci in enumerate((c0, c1, c2, c3)):
            c, half = divmod(i, 2)
            aw[:, c, half * D:(half + 1) * D, :S] = xt * di[None, None, :]
            aw[:, c, half * D:(half + 1) * D, S:] = ci * yi
            di = di * d
            yi = yi * yo_r
    return aw, n_chunks, wk


def kernel(**inputs):
    x = np.asarray(inputs["x"], dtype=np.float32)
    y = np.asarray(inputs["y"], dtype=np.float32)
    dm = np.asarray(inputs["decay_mask"], dtype=np.float32)
    qk = int(np.asarray(inputs["QK_mul"]))

    aw, n_chunks, wk = _prepare(x, y, dm, qk)
    nc = _get_nc(n_chunks, wk)

    in_maps = [
        {"aw": aw[c * BLK:(c + 1) * BLK]} for c in range(N_CORES)
    ]
    global _last_nc, _last_in_maps
    _last_nc, _last_in_maps = nc, in_maps

    res = None
    for attempt in range(3):
        try:
            res = run_bass_kernel_spmd(nc, in_maps,
                                       core_ids=list(range(N_CORES)))
            break
        except Exception:
            # transient NRT_EXEC_UNIT_UNRECOVERABLE wedges occur on busy axon
            # terminals; they clear after a pause
            if attempt == 2:
                raise
            import time
            time.sleep(45)

    out = np.empty((BH, S, S), dtype=np.float32)
    for c in range(N_CORES):
        out[c * BLK:(c + 1) * BLK] = res.results[c]["out"]
    return out.reshape(B, H, S, S)


# revision 31
# speedup vs baseline: 1.0749x; 1.0014x over previous
"""Trainium2 Bass kernel for nn_DRAM_MAC_temporal_encoding (polynomial attention).

Math (QK_mul=1):
    out = sum_i coef_i * (x @ (y-OFF)^i) * decay
        = (x * decay) @ P(y-OFF)            # P = Horner cubic, elementwise
so the whole problem is ONE [S,64]@[64,S] matmul per (b,h) head plus the
output write -> memory-bound. The tiny elementwise prep (poly on y,
row-scaling x, transposes, fp16 casts) runs on host; the device does
matmuls + store.

Precision: tolerance is rel_err < 2e-2. fp16 inputs + single fp16 matmul
(fp32 PSUM accumulate) + fp16 output measures 2.5e-4 on the numpy model —
so no hi/lo split and, crucially, the 50 MiB/core fp32 output write
becomes 25 MiB fp16 (host upcasts back to fp32). PSUM->SBUF fp32->fp16
drains rotate across Vector/Scalar/Pool so no single engine bottlenecks.

QK_mul=0: out = sum_i coef_i * ((x*d^i) @ (y-OFF)^i) -> two K=128 chunks
(4 stacked K=64 terms), same kernel with n_chunks=2.

Sharding: 24 (b,h) heads -> 3 per core across 8 cores.
"""

import ml_dtypes
import numpy as np

import concourse.mybir as mybir
import concourse.tile as tile
from concourse import bacc
from concourse.bass_utils import run_bass_kernel_spmd

C = [0.17393044, 0.15653739, 0.14088365, 0.12679529, 5.51975209,
     4.96777688, 4.4709992, -1.44776001, -1.30298401, 46.05483778]
MAX_ORDER = 3
X_MAX = 0.9
OFFSET = 0.45

B, H, S, D = 2, 12, 2048, 64
BH = B * H
N_CORES = 8
BLK = BH // N_CORES  # heads per core

M_TILE = 128   # output rows per matmul (PSUM partitions)
N_TILE = 512   # output cols per matmul (one fp32 PSUM bank)

_NC_CACHE = {}
_last_nc = None
_last_in_maps = None


def _coefs():
    cs = []
    idx = 0
    for i in range(MAX_ORDER + 1):
        n_j = MAX_ORDER - i + 1
        cs.append(sum(C[idx + j] * X_MAX ** j for j in range(n_j)))
        idx += n_j
    return cs  # [c0, c1, c2, c3]


def _build_nc(n_chunks, wk):
    """Device kernel: per core, BLK independent [S,S] fp16 output blocks,
    each output tile = sum over n_chunks K=128 bf16 matmuls.

    K=64 matmuls stream at ~1/3 the K=128 rate on TRN2 HW (630ns vs 233ns
    per [128,512]), so the contraction is always presented as K=128. Head 0
    uploads full pre-padded 128-row operands ([a_hi; a_lo] and [w; w]) so
    its first matmul isn't gated by the ~1.8us-per-tile Pool memsets; heads
    1+ upload only 64 real rows and zero rows 64:128 on the (otherwise
    idle) Pool engine well before they're needed."""
    nc = bacc.Bacc(None, target_bir_lowering=False)
    aw_d = nc.dram_tensor("aw", [BLK, n_chunks, wk, 2 * S],
                          mybir.dt.bfloat16, kind="ExternalInput")
    out_d = nc.dram_tensor("out", [BLK, S, S], mybir.dt.float16,
                           kind="ExternalOutput")

    with tile.TileContext(nc) as tc:
        with (
            tc.tile_pool(name="inp", bufs=1) as inp,
            tc.tile_pool(name="ps", bufs=4, space="PSUM") as psp,
            tc.tile_pool(name="outp", bufs=14) as outp,
        ):
            # Input tiles: zero rows wk:128 are memset up front (blk0's w on
            # DVE, which is idle until drains start; the rest on Pool), and
            # each head's loads are emitted just before its row-tiles so the
            # single DMA FIFO starts storing after only one head's loads.
            aw_ts = {}
            for blk in range(BLK):
                for c in range(n_chunks):
                    t = inp.tile([128, 2 * S], mybir.dt.bfloat16,
                                 name=f"aw{blk}_{c}", tag=f"aw{blk}_{c}")
                    aw_ts[(blk, c)] = t
            if wk < 128:
                for blk in range(BLK):
                    for c in range(n_chunks):
                        if blk == 0:
                            # head 0 gates the pipeline start: its zero rows
                            # go to DVE/Act (idle until the first drains,
                            # and 2x-mode-fast on bf16) instead of Pool
                            nc.vector.memset(aw_ts[(blk, c)][wk:, :S], 0.0)
                            nc.scalar.memzero(aw_ts[(blk, c)][wk:, S:])
                        else:
                            nc.gpsimd.memset(aw_ts[(blk, c)][wk:], 0.0)

            # Pool/GpSimd can't read PSUM on TRN2, so drains go to DVE and
            # Act. A 2-deep ring of [128,2048] PSUM tiles serializes on the
            # ~2.2us whole-tile drain (measured ~2.0us/row-tile cadence);
            # instead use a 4-deep ring of [128,1024] half-tiles whose
            # ~1.1us drains alternate engines.
            HALF = S // 2
            di = 0
            with nc.allow_low_precision(reason="fp16 out within 2e-2 tol"):
                for blk in range(BLK):
                    for c in range(n_chunks):
                        nc.sync.dma_start(aw_ts[(blk, c)][:wk],
                                          aw_d[blk, c])
                    for st in range(S // M_TILE):
                        ot = outp.tile([M_TILE, S], mybir.dt.float16,
                                       tag="ot")
                        for h in range(2):
                            ps = psp.tile([M_TILE, HALF], mybir.dt.float32,
                                          tag="ps")
                            for ntl in range(HALF // N_TILE):
                                nt = h * (HALF // N_TILE) + ntl
                                for c in range(n_chunks):
                                    nc.tensor.matmul(
                                        ps[:, ntl * N_TILE:
                                           (ntl + 1) * N_TILE],
                                        aw_ts[(blk, c)][
                                            :, st * M_TILE:(st + 1) * M_TILE],
                                        aw_ts[(blk, c)][
                                            :, S + nt * N_TILE:
                                            S + (nt + 1) * N_TILE],
                                        start=(c == 0),
                                        stop=(c == n_chunks - 1),
                                    )
                            dst = ot[:, h * HALF:(h + 1) * HALF]
                            if di % 2 == 0:
                                nc.vector.tensor_copy(dst, ps[:])
                            else:
                                nc.scalar.copy(dst, ps[:])
                            di += 1
                        nc.sync.dma_start(
                            out_d[blk, st * M_TILE:(st + 1) * M_TILE, :],
                            ot[:])
    nc.compile()
    return nc


def _get_nc(n_chunks, wk):
    key = (n_chunks, wk)
    if key not in _NC_CACHE:
        _NC_CACHE[key] = _build_nc(n_chunks, wk)
    return _NC_CACHE[key]


def _prepare(x, y, dm, qk):
    """Host prep -> bf16 arrays: aw [BH, n_chunks, wk, 2S] (a|w column
    blocks, heads 1+ of each core) and aw0p [N_CORES, n_chunks, 128, 2S]
    (pre-padded head 0 of each core: [a_hi; a_lo] | [w; w])."""
    c0, c1, c2, c3 = _coefs()
    yo = (y - OFFSET).astype(np.float32)  # [B,H,D,S]
    if qk:
        n_chunks, wk = 1, D
        af = np.ascontiguousarray(
            (x * dm[None, None, :, :]).transpose(0, 1, 3, 2)
        ).reshape(BH, 1, D, S).astype(np.float32)
        aw = np.empty((BH, 1, D, 2 * S), dtype=ml_dtypes.bfloat16)
        aw[..., :S] = af
        aw[..., S:] = (((c3 * yo + c2) * yo + c1) * yo + c0) \
            .astype(ml_dtypes.bfloat16).reshape(BH, 1, D, S)
    else:
        n_chunks, wk = 2, 2 * D
        d = dm[:, 0]
        aw = np.empty((BH, 2, 2 * D, 2 * S), dtype=ml_dtypes.bfloat16)
        xt = x.transpose(0, 1, 3, 2).reshape(BH, D, S)
        di = np.ones_like(d)
        yi = np.ones_like(yo).reshape(BH, D, S)
        yo_r = yo.reshape(BH, D, S)
        for i, ci in enumerate((c0, c1, c2, c3)):
            c, half = divmod(i, 2)
            aw[:, c, half * D:(half + 1) * D, :S] = xt * di[None, None, :]
            aw[:, c, half * D:(half + 1) * D, S:] = ci * yi
            di = di * d
            yi = yi * yo_r
    return aw, n_chunks, wk


def kernel(**inputs):
    x = np.asarray(inputs["x"], dtype=np.float32)
    y = np.asarray(inputs["y"], dtype=np.float32)
    dm = np.asarray(inputs["decay_mask"], dtype=np.float32)
    qk = int(np.asarray(inputs["QK_mul"]))

    aw, n_chunks, wk = _prepare(x, y, dm, qk)
    nc = _get_nc(n_chunks, wk)

    in_maps = [
        {"aw": aw[c * BLK:(c + 1) * BLK]} for c in range(N_CORES)
    ]
    global _last_nc, _last_in_maps
    _last_nc, _last_in_maps = nc, in_maps

    res = None
    for attempt in range(3):
        try:
            res = run_bass_kernel_spmd(nc, in_maps,
                                       core_ids=list(range(N_CORES)))
            break
        except Exception:
            # transient NRT_EXEC_UNIT_UNRECOVERABLE wedges occur on busy axon
            # terminals; they clear after a pause
            if attempt == 2:
                raise
            import time
            time.sleep(45)

    out = np.empty((BH, S, S), dtype=np.float32)
    for c in range(N_CORES):
        out[c * BLK:(c + 1) * BLK] = res.results[c]["out"]
    return out.reshape(B, H, S, S)
